# revision 60
# baseline (speedup 1.0000x reference)
"""Cantor-route sparse attention on 8 Trainium2 NeuronCores.

Strategy
--------
The routes table (top-16 nearest neighbors by Cantor coordinate) depends only
on T=4096, so it is computed on the host. Sorting positions by the Cantor
coordinate makes the gather nearly block-diagonal: every block of 256
rank-sorted queries draws its 16-per-query routed keys from a union of at most
249 distinct positions. Each such block therefore becomes a dense 256x256
masked attention against a host-gathered key buffer of 256 rows.

Sharding: 8 cores = batch (2) x rank-chunks (4 x 1024 queries). Each core
computes Q/K/V projections for its own gathered rows, 4 blocks x 8 heads of
masked softmax attention, and the output projection for its 1024 queries.
No cross-core communication; the host scatters rows back.

Softmax: the routed-slot mask {0, -1e9} is added on the PE via an accumulating
identity matmul; exp reads scores straight from PSUM with a constant -25 bias
(ACT bias operand) keeping exp finite (score max 98.6, row-max min -1.22).
Sums come from a ones-column appended to V (the attV matmul emits row sums);
normalization = DVE reciprocal + GpSimd partition-broadcast + DVE mul.

The program is split into one-time setup (weights, mask, identity — loaded
once, resident in SBUF) and a per-iteration compute body; under For_i timing
loops only x DMAs, projections, attention and y DMAs repeat. qt/kt/vones are
double-buffered (phase-alternating loop body) and the y output blocks are
software-pipelined: y(0)/y(1)/y(2) are emitted INSIDE later attention
stretches and y(3) is deferred into the next iteration, so the in-order PE
always has ready matmul work while the exp/normalize chains drain. The two
heads of a pair emit their K=64 score matmuls interleaved: kt slices at base
partitions 0/64 land in disjoint PE row-groups and overlap in hardware.
Engine balance: exp + q/v epilogues + ys on ACT, k epilogue + normalize on
DVE, broadcast on Pool (GPSIMD cannot touch PSUM), mask-add on PE.

Precision: matmul operands are float32r (PE runs 1 cycle/row vs 4 for plain
fp32) except the value path (exp output and V) in bf16; accumulation stays
fp32 in PSUM. Biases are folded into the matmuls as ones-row rank-1 updates.
Note: reciprocal_approx_fast mis-reads partition-offset inputs (sum row lives
at PSUM partition 64), so the exact DVE reciprocal is used.
"""

import math
import numpy as np
import ml_dtypes
from contextlib import ExitStack

# ---- problem constants (hardcoded; kernel.py must be self-contained) ----
B, T, D = 2, 4096, 512
H, HD, W = 8, 64, 16
DEPTH = 8
NCORES = 8
QCHUNK = 1024          # queries per core (rank space)
BLK = 256              # queries per attention block
KB = 256               # key slots per block
NBLK = QCHUNK // BLK   # blocks per core
NG = T // BLK          # global blocks
E_BIAS = -25.0         # constant exp bias keeping exp(s+E_BIAS) finite

_routing_cache = None
_program_cache = {}


def _build_routing():
    """Host-side replication of reference routes + rank-space layout."""
    global _routing_cache
    if _routing_cache is not None:
        return _routing_cache
    pos = np.arange(T, dtype=np.float32)
    x = np.clip(pos / np.float32(T - 1), np.float32(1e-6),
                np.float32(1.0 - 1e-6)).astype(np.float32)
    c = np.zeros_like(x)
    factor = np.float32(0.5)
    for _ in range(DEPTH):
        x = (x * np.float32(3.0)).astype(np.float32)
        digit = np.floor(x)
        x = (x - digit).astype(np.float32)
        c = (c + factor * (digit == np.float32(2.0)).astype(np.float32)).astype(np.float32)
        factor = np.float32(factor * np.float32(0.5))
    dist = np.abs(c[:, None] - c[None, :])
    # jax.lax.top_k(-dist): smallest distances, ties -> lower index
    routes = np.argsort(dist, axis=1, kind="stable")[:, :W].astype(np.int64)
    perm = np.argsort(c, kind="stable")          # rank -> position
    rank = np.empty(T, np.int64)
    rank[perm] = np.arange(T)
    route_ranks = rank[routes]                   # [T(pos), W]

    keylist = np.zeros((NG, KB), np.int64)       # ranks of key slots
    maskT = np.full((NG, KB, BLK), np.float32(-1e9), np.float32)
    for g in range(NG):
        qranks = np.arange(g * BLK, (g + 1) * BLK)
        rr = route_ranks[perm[qranks]]           # [BLK, W]
        kr = np.unique(rr)
        n = len(kr)
        assert n <= KB, f"block {g} union {n} > {KB}"
        keylist[g, :n] = kr
        keylist[g, n:] = kr[-1]
        hit = (keylist[g][:n, None, None] == rr[None, :, :]).any(-1)  # [n, BLK]
        blk_mask = maskT[g]
        sub = blk_mask[:n, :]
        sub[hit] = 0.0
        blk_mask[:n, :] = sub
        assert ((blk_mask == 0.0).sum(0) == W).all()
    _routing_cache = (perm, keylist, maskT)
    return _routing_cache


def _build_program(loop_n=1, with_bias=False, unroll=1):
    """Build + compile the SPMD Bass program (cached per loop_n)."""
    key = (loop_n, with_bias, unroll)
    if key in _program_cache:
        return _program_cache[key]
    import concourse.tile as tile
    from concourse import bacc, mybir
    from concourse.masks import make_identity

    f32 = mybir.dt.float32
    f32r = mybir.dt.float32r
    bf16 = mybir.dt.bfloat16
    nc = bacc.Bacc("TRN2", target_bir_lowering=False, debug=False,
                   enable_asserts=False, num_devices=NCORES)

    NKV = NBLK * KB            # 1024 gathered key rows
    KT_TILES = NKV // 128      # 8 key partition tiles

    # f32r inputs feed matmuls directly (PE rounds on read; verified on HW)
    d_xqT = nc.dram_tensor("xqT", [D, QCHUNK], f32r, kind="ExternalInput").ap()
    d_xkvT = nc.dram_tensor("xkvT", [D, NKV], f32r, kind="ExternalInput").ap()
    d_wq = nc.dram_tensor("wq", [D, D], f32r, kind="ExternalInput").ap()
    d_wk = nc.dram_tensor("wk", [D, D], f32r, kind="ExternalInput").ap()
    d_wv = nc.dram_tensor("wv", [D, D], f32r, kind="ExternalInput").ap()
    d_wo = nc.dram_tensor("wo", [D, D], f32r, kind="ExternalInput").ap()
    if with_bias:
        d_bqT = nc.dram_tensor("bqT", [128, 4], f32, kind="ExternalInput").ap()
        d_bkT = nc.dram_tensor("bkT", [128, 4], f32, kind="ExternalInput").ap()
        d_bvr = nc.dram_tensor("bvr", [1, D], f32r, kind="ExternalInput").ap()
        d_bor = nc.dram_tensor("bor", [1, D], f32r, kind="ExternalInput").ap()
    d_maskT = nc.dram_tensor("maskT", [NKV, BLK], bf16, kind="ExternalInput").ap()
    d_y = nc.dram_tensor("y", [QCHUNK, D], f32, kind="ExternalOutput").ap()

    def build(tc, ctx):
        """One-time setup (pools, constants, weights, mask); returns the
        per-iteration compute() closure."""
        consts = ctx.enter_context(tc.tile_pool(name="consts", bufs=1))
        sb = ctx.enter_context(tc.tile_pool(name="sb", bufs=1))

        # ---- tiny DMAs + constants first (not queued behind the big ones) ----
        if with_bias:
            bqT = consts.tile([128, 4], f32, tag="bqT")
            nc.sync.dma_start(bqT[:], d_bqT[:, :])
            bkT = consts.tile([128, 4], f32, tag="bkT")
            nc.sync.dma_start(bkT[:], d_bkT[:, :])
            bvr = consts.tile([1, D], f32r, tag="bvr")
            nc.sync.dma_start(bvr[0:1, :], d_bvr[:, :])
            bor = consts.tile([1, D], f32r, tag="bor")
            nc.sync.dma_start(bor[0:1, :], d_bor[:, :])
            ones_f = consts.tile([128, 128], f32, tag="ones_f")
            nc.gpsimd.memset(ones_f[:], 1.0)
            ones = consts.tile([128, 128], f32r, tag="ones")
            nc.gpsimd.tensor_copy(ones[:], ones_f[:])
        ebias = consts.tile([128, 1], f32, tag="ebias")
        nc.gpsimd.memset(ebias[:], E_BIAS)
        ones_b = consts.tile([128, H], bf16, tag="ones_b")
        nc.gpsimd.memset(ones_b[:], 1.0)
        ident_f = consts.tile([128, 128], f32, tag="ident_f")
        make_identity(nc, ident_f[:])
        ident = consts.tile([128, 128], bf16, tag="ident")
        nc.gpsimd.tensor_copy(ident[:], ident_f[:])

        # ---- weights + mask: loaded once, stay resident across iterations ----
        def load_wide(name, dram, rows, cols, dt):
            t = consts.tile([128, (rows // 128) * cols], dt, tag=name, name=name)
            nc.sync.dma_start(
                t.rearrange("p (c n) -> p c n", n=cols),
                dram.rearrange("(c p) n -> p c n", p=128))
            return t

        wq_t = load_wide("wq", d_wq, D, D, f32r)
        wk_t = load_wide("wk", d_wk, D, D, f32r)
        wv_t = load_wide("wv", d_wv, D, D, f32r)
        wo_t = load_wide("wo", d_wo, D, D, f32r)
        mk_t = consts.tile([128, 2 * NBLK * BLK], bf16, tag="mk", name="mk")
        for half in range(2):
            nc.sync.dma_start(
                mk_t[:, half * 4 * BLK:(half + 1) * 4 * BLK].rearrange(
                    "p (c n) -> p c n", n=BLK),
                d_maskT[half * 512:(half + 1) * 512, :].rearrange(
                    "(c p) n -> p c n", p=128))

        xq_t = consts.tile([128, 4 * QCHUNK], f32r, tag="xq", name="xq")
        xkv_t = consts.tile([128, 4 * NKV], f32r, tag="xkv", name="xkv")

        def dma_xq(xh, w=512):
            for a in range(xh * 512, (xh + 1) * 512, w):
                nc.sync.dma_start(
                    xq_t.rearrange("p (c n) -> p c n", n=QCHUNK)[
                        :, :, a:a + w],
                    d_xqT.rearrange("(c p) n -> p c n", p=128)[
                        :, :, a:a + w])

        def dma_xkv(xh):
            nc.sync.dma_start(
                xkv_t.rearrange("p (c n) -> p c n", n=NKV)[
                    :, :, xh * 512:(xh + 1) * 512],
                d_xkvT.rearrange("(c p) n -> p c n", p=128)[
                    :, :, xh * 512:(xh + 1) * 512])

        def wsl(t, dc, a, b):        # weight slice [128, b-a] of chunk dc
            return t[:, dc * D + a: dc * D + b]

        def xsl(t, n, dc, a, b):     # x slice of chunk dc
            return t[:, dc * n + a: dc * n + b]

        # ---- persistent activation tiles ----
        # qt/kt/vones are double-buffered (phase 0/1): consecutive loop
        # iterations alternate sets so iteration i+1's projections don't
        # wait for iteration i's last attention reads.
        qt_sbs = [[sb.tile([128, QCHUNK], f32r, tag=f"qt{ph}_{fc}",
                           name=f"qt{ph}_{fc}") for fc in range(4)]
                  for ph in range(2)]
        kt_sbs = [[sb.tile([128, NKV], f32r, tag=f"kt{ph}_{fc}",
                           name=f"kt{ph}_{fc}") for fc in range(4)]
                  for ph in range(2)]
        voness = [[sb.tile([128, H * (HD + 1)], bf16, tag=f"vones{ph}_{i}",
                           name=f"vones{ph}_{i}") for i in range(KT_TILES)]
                  for ph in range(2)]
        attT = [[sb.tile([128, BLK], f32r, tag=f"attT{bl}_{hp}",
                         name=f"attT{bl}_{hp}")
                 for hp in range(4)] for bl in range(NBLK)]
        cur = {}

        ps_big = ctx.enter_context(
            tc.tile_pool(name="ps_big", bufs=2, space="PSUM"))
        ps_s = ctx.enter_context(
            tc.tile_pool(name="ps_s", bufs=3, space="PSUM"))
        ps_o = ctx.enter_context(
            tc.tile_pool(name="ps_o", bufs=3, space="PSUM"))
        work = ctx.enter_context(tc.tile_pool(name="work", bufs=8))
        wsm = ctx.enter_context(tc.tile_pool(name="wsm", bufs=4))

        if True:
            def proj_q(fc, qt, nw=512):
                ps = ps_big.tile([128, 512], f32, tag="big")
                for sub in range(0, 512, nw):
                    for dc in range(4):
                        nc.tensor.matmul(
                            ps[:, sub:sub + nw],
                            lhsT=wsl(wq_t, dc, fc * 128, (fc + 1) * 128),
                            rhs=xsl(xq_t, QCHUNK, dc,
                                    qt * 512 + sub, qt * 512 + sub + nw),
                            start=(dc == 0), stop=(dc == 3))
                if with_bias:
                    nc.scalar.activation(
                        cur['qt'][fc][:, qt * 512:(qt + 1) * 512], ps[:],
                        mybir.ActivationFunctionType.Identity,
                        bias=bqT[:, fc:fc + 1], scale=1.0)
                else:
                    nc.scalar.copy(
                        cur['qt'][fc][:, qt * 512:(qt + 1) * 512], ps[:])

            def proj_k(fc, qt):
                ps = ps_big.tile([128, 512], f32, tag="big")
                for dc in range(4):
                    nc.tensor.matmul(
                        ps[:],
                        lhsT=wsl(wk_t, dc, fc * 128, (fc + 1) * 128),
                        rhs=xsl(xkv_t, NKV, dc, qt * 512, (qt + 1) * 512),
                        start=(dc == 0), stop=(dc == 3))
                if with_bias:
                    nc.vector.tensor_scalar_add(
                        cur['kt'][fc][:, qt * 512:(qt + 1) * 512], ps[:],
                        bkT[:, fc:fc + 1])
                else:
                    nc.vector.tensor_copy(
                        cur['kt'][fc][:, qt * 512:(qt + 1) * 512], ps[:])

            def proj_v(kt):
                nc.gpsimd.tensor_copy(
                    cur['vones'][kt].rearrange("p (h e) -> p h e",
                                        h=H)[:, :, HD:HD + 1],
                    ones_b.rearrange("p (h e) -> p h e", e=1))
                ps = ps_big.tile([128, 512], f32, tag="big")
                for dc in range(4):
                    nc.tensor.matmul(
                        ps[:],
                        lhsT=xsl(xkv_t, NKV, dc, kt * 128, (kt + 1) * 128),
                        rhs=wsl(wv_t, dc, 0, D),
                        start=(dc == 0), stop=(not with_bias and dc == 3))
                if with_bias:
                    nc.tensor.matmul(
                        ps[:], lhsT=ones[0:1, 0:128], rhs=bvr[0:1, :],
                        start=False, stop=True)
                nc.scalar.activation(
                    cur['vones'][kt].rearrange("p (h e) -> p h e", h=H)[:, :, 0:HD],
                    ps.rearrange("p (h e) -> p h e", h=H),
                    mybir.ActivationFunctionType.Copy)

            def attn_headpair(bl, hp):
                # pass A: scores+exp for BOTH heads first — PE streams head
                # 1's score matmuls while head 0's exp runs, so the attnV
                # matmuls in pass B never wait on ACT
                etr2 = []
                # masks first (full-array matmuls), then score pieces
                # interleaved across the two heads: hr0 loads kt rows 0-63
                # (PE row groups 0-1), hr1 rows 64-127 (groups 2-3) — the
                # implicit tile_position lets the array overlap them
                pss2 = [ps_s.tile([128, 2 * BLK], f32, tag="sT", name="sT")
                        for _ in range(2)]
                for hr in range(2):
                    nc.tensor.matmul(
                        pss2[hr][:],
                        lhsT=ident[:, :],
                        rhs=mk_t[:, (bl * 2) * BLK:(bl * 2 + 2) * BLK],
                        start=True, stop=False)
                for piece in range(2):
                    for hr in range(2):
                        nc.tensor.matmul(
                            pss2[hr][:, piece * BLK:(piece + 1) * BLK],
                            lhsT=cur['kt'][hp][hr * 64:hr * 64 + 64,
                                           bl * BLK + piece * 128:
                                           bl * BLK + piece * 128 + 128],
                            rhs=cur['qt'][hp][hr * 64:hr * 64 + 64,
                                          bl * BLK:(bl + 1) * BLK],
                            start=False, stop=(piece == 1))
                for hr in range(2):
                    etr = work.tile([128, 2 * BLK], bf16, tag="eTr")
                    nc.scalar.activation(
                        etr[:], pss2[hr][:],
                        mybir.ActivationFunctionType.Exp,
                        bias=ebias[:, 0:1], scale=1.0)
                    etr2.append(etr)
                # pass B: attnV + normalize
                pso2 = []
                invs = wsm.tile([1, 2 * BLK], f32, tag="invs")
                for hr in range(2):
                    h = hp * 2 + hr
                    pso = ps_o.tile([HD + 1, BLK], f32, tag="oT")
                    for piece in range(2):
                        nc.tensor.matmul(
                            pso[:],
                            lhsT=cur['vones'][bl * 2 + piece][
                                :, h * (HD + 1):(h + 1) * (HD + 1)],
                            rhs=etr2[hr][:, piece * BLK:(piece + 1) * BLK],
                            start=(piece == 0), stop=(piece == 1))
                    pso2.append(pso)
                    nc.vector.reciprocal(
                        invs[0:1, hr * BLK:(hr + 1) * BLK],
                        pso[HD:HD + 1, :])
                inv_sb = wsm.tile([HD, 2 * BLK], f32, tag="inv_sb")
                nc.gpsimd.partition_broadcast(inv_sb[:], invs[0:1, :])
                for hr in range(2):
                    nc.vector.tensor_mul(
                        attT[bl][hp][hr * 64:hr * 64 + 64, :],
                        pso2[hr][0:HD, :],
                        inv_sb[:, hr * BLK:(hr + 1) * BLK])

            def y_block(bl):
                for sub in range(2):
                    psy = ps_big.tile([128, D], f32, tag="big")
                    for fc in range(4):
                        nc.tensor.matmul(
                            psy[:],
                            lhsT=attT[bl][fc][:, sub * 128:(sub + 1) * 128],
                            rhs=wsl(wo_t, fc, 0, D),
                            start=(fc == 0), stop=(not with_bias and fc == 3))
                    if with_bias:
                        nc.tensor.matmul(
                            psy[:], lhsT=ones[0:1, 0:128], rhs=bor[0:1, :],
                            start=False, stop=True)
                    ys = wsm.tile([128, D], f32, tag="ys")
                    nc.scalar.copy(ys[:], psy[:])
                    nc.sync.dma_start(
                        d_y[bl * BLK + sub * 128: bl * BLK + sub * 128 + 128,
                            :], ys[:])

            def compute(ph, first):
                # interleave: half-1 projections are emitted inside half-0's
                # attention so the scheduler can fill PE gaps at the
                # transition. y(2)/y(3) of the PREVIOUS phase are emitted at
                # the top (software pipelining): their attT inputs are long
                # ready, so they fill the x-DMA wait window and remove the
                # tail stall behind attn(3)'s normalize chain.
                cur["qt"] = qt_sbs[ph]
                cur["kt"] = kt_sbs[ph]
                cur["vones"] = voness[ph]
                dma_xq(0, w=256)
                dma_xkv(0)
                dma_xq(1)
                dma_xkv(1)
                for fc in range(4):
                    proj_q(fc, 0, nw=256)
                for fc in range(4):
                    proj_k(fc, 0)
                if not first:
                    y_block(3)
                for kt in range(4):
                    proj_v(kt)
                for hp in range(4):
                    attn_headpair(0, hp)
                    proj_q(hp, 1)
                for hp in range(2):
                    attn_headpair(1, hp)
                    proj_k(hp, 1)
                y_block(0)
                for hp in range(2, 4):
                    attn_headpair(1, hp)
                    proj_k(hp, 1)
                proj_v(4)
                proj_v(5)
                attn_headpair(2, 0)
                y_block(1)
                proj_v(6)
                attn_headpair(2, 1)
                proj_v(7)
                for hp in range(2, 4):
                    attn_headpair(2, hp)
                attn_headpair(3, 0)
                attn_headpair(3, 1)
                y_block(2)
                attn_headpair(3, 2)
                attn_headpair(3, 3)

            def flush():
                y_block(3)

            return compute, flush

    with tile.TileContext(nc) as tc, ExitStack() as ctx:
        compute, flush = build(tc, ctx)
        if loop_n == 1:
            for i in range(unroll):
                compute(i % 2, first=(i == 0))
            flush()
        else:
            # two phase-alternating iterations per hardware-loop trip so the
            # double-buffered activation tiles actually alternate; the
            # deferred y(2)/y(3) of the previous trip's phase-1 are emitted
            # at the top of the next trip (loop-carried attT dependency)
            assert loop_n % 2 == 0, "loop_n must be even"
            with tc.For_i(0, loop_n // 2, 1):
                compute(0, first=False)
                compute(1, first=False)
            flush()
    nc.compile()
    _program_cache[key] = nc
    return nc


def _prep_core_inputs(inputs):
    """Host-side shard prep: returns in_maps (list of 8 dicts) + scatter info."""
    perm, keylist, maskT = _build_routing()
    x = np.ascontiguousarray(np.asarray(inputs["x"], dtype=np.float32))
    temp = np.float32(inputs["temperature"])
    scale = np.float32(math.sqrt(HD) * abs(float(temp)))
    wq = (np.asarray(inputs["Wq"], np.float32) / scale).astype(np.float32)
    bq = (np.asarray(inputs["bq"], np.float32) / scale).astype(np.float32)
    wk = np.ascontiguousarray(np.asarray(inputs["Wk"], np.float32))
    bk = np.asarray(inputs["bk"], np.float32)
    wv = np.ascontiguousarray(np.asarray(inputs["Wv"], np.float32))
    bv = np.asarray(inputs["bv"], np.float32)
    wo = np.ascontiguousarray(np.asarray(inputs["Wo"], np.float32))
    bo = np.asarray(inputs["bo"], np.float32)

    bqT = np.ascontiguousarray(bq.reshape(4, 128).T)
    bkT = np.ascontiguousarray(bk.reshape(4, 128).T)

    in_maps = []
    qpos_per_core = []
    for core in range(NCORES):
        b, j = divmod(core, NBLK)
        qranks = np.arange(j * QCHUNK, (j + 1) * QCHUNK)
        qpos = perm[qranks]
        gs = slice(NBLK * j, NBLK * j + NBLK)
        keypos = perm[keylist[gs].reshape(-1)]
        in_maps.append({
            "xqT": np.ascontiguousarray(x[b, qpos].T),
            "xkvT": np.ascontiguousarray(x[b, keypos].T),
            "wq": wq, "wk": wk, "wv": wv, "wo": wo,
            "bqT": bqT, "bkT": bkT,
            "bvr": bv.reshape(1, D), "bor": bo.reshape(1, D),
            "maskT": np.ascontiguousarray(
                maskT[gs].reshape(NBLK * KB, BLK)).astype(
                    ml_dtypes.bfloat16),
        })
        qpos_per_core.append((b, qpos))
    return in_maps, qpos_per_core


def kernel(**inputs):
    from concourse.bass_utils import run_bass_kernel_spmd
    with_bias = not all(
        float(np.abs(np.asarray(inputs[k])).max()) == 0.0
        for k in ("bq", "bk", "bv", "bo"))
    nc = _build_program(loop_n=1, with_bias=with_bias)
    in_maps, qpos_per_core = _prep_core_inputs(inputs)
    if not with_bias:
        drop = {"bqT", "bkT", "bvr", "bor"}
        in_maps = [{k: v for k, v in m.items() if k not in drop}
                   for m in in_maps]
    res = run_bass_kernel_spmd(nc, in_maps, core_ids=list(range(NCORES)))
    out = np.zeros((B, T, D), np.float32)
    for core in range(NCORES):
        b, qpos = qpos_per_core[core]
        out[b, qpos] = res.results[core]["y"]
    return out

